# revision 12
# baseline (speedup 1.0000x reference)
"""Tensor-parallel GQA attention prefill for 8 TRN2 NeuronCores.

Sharding: each core owns 4 query heads + 1 kv head (column-shard of
wq/wk/wv by head) and a 512-row slice of wo's input dim (row-shard).
Each core computes a partial output projection over its local heads;
the host sums the 8 partials (the all-reduce) and transposes back.

Fast path (bf16) structure, tuned from perfetto/NTFF traces:
  - warmup matmuls on a junk tile (no gpsimd memset) to lift the HAM
    clock gate while DMAs land
  - K+V sweep first (needs only wk+wv), riding the x stream; Q-head
    sweeps follow from SBUF-resident x. Weights for later sweeps are
    queued behind x; wq2/wq3/wo go via the gpsimd SWDGE queue so the
    two HWDGE rings carry only the critical stream and the scalar
    engine is not credit-blocked when rope/softmax work starts.
  - scores computed TRANSPOSED (keys on partitions): softmax denom is
    a ones-vector matmul (partition reduction on PE), no P-transpose,
    normalization folded into the oT eviction via a PE-broadcast of
    1/den. No max-subtraction (fast path gates sigma < 8, exp stays
    comfortably inside fp32).
  - O-projection starts early on heads 0/1 for the first psum banks
    while the last head's softmax drains.
"""

import math
from contextlib import ExitStack

import ml_dtypes
import numpy as np

import concourse.bass as bass
import concourse.tile as tile
from concourse import bacc, mybir
from concourse.bass_utils import run_bass_kernel_spmd

DIM = 4096
N_HEADS = 32
HEAD_DIM = 128
N_KV_HEADS = 8
BSZ = 4
SEQLEN = 128
T = BSZ * SEQLEN  # 512 tokens
NCORES = 8
HQ = N_HEADS // NCORES  # 4 query heads per core
EQ = HQ * HEAD_DIM  # 512 local q features
ND = DIM // 128  # 32 contraction tiles
SCALE = 1.0 / math.sqrt(HEAD_DIM)

F32 = mybir.dt.float32
F32R = mybir.dt.float32r
BF16 = mybir.dt.bfloat16
AX = mybir.AxisListType
ACTF = mybir.ActivationFunctionType
PSUM = bass.MemorySpace.PSUM

_STATE: dict = {}
LAST_RESULT = None


def _install_ntff_hook():
    """Register the axon NTFF profile hook if the image lacks antenv.axon_hooks."""
    import os
    import sys
    import types

    try:
        import antenv.axon_hooks  # noqa: F401

        return
    except ImportError:
        pass
    try:
        import antenv
        from trn_agent_boot.trn_boot import _ntff_profile_via_ctypes

        mod = types.ModuleType("antenv.axon_hooks")
        holder = {"hook": None}
        mod.set_axon_ntff_profile_hook = lambda h: holder.__setitem__("hook", h)
        mod.get_axon_ntff_profile_hook = lambda: holder["hook"]
        sys.modules["antenv.axon_hooks"] = mod
        antenv.axon_hooks = mod
        so = "/opt/axon/libaxon_pjrt.so"
        if os.path.exists(so):
            hook = _ntff_profile_via_ctypes(so)
            if hook is not None:
                mod.set_axon_ntff_profile_hook(hook)
    except Exception:
        pass


_install_ntff_hook()

XGROUPS = [2, 2, 2, 2, 4, 4, 4, 4, 4, 4]
XG_COL = []
_j0 = 0
for _gd in XGROUPS:
    XG_COL.append((_j0, _gd))
    _j0 += _gd


def _build_nc_fast():
    nc = bacc.Bacc(
        "TRN2",
        target_bir_lowering=False,
        debug=False,
        enable_asserts=False,
        num_devices=NCORES,
    )
    xT = nc.dram_tensor("xT", [128, ND * T], BF16, kind="ExternalInput").ap()
    wqT = nc.dram_tensor("wqT", [128, HQ * ND * HEAD_DIM], BF16, kind="ExternalInput").ap()
    wkT = nc.dram_tensor("wkT", [128, ND * HEAD_DIM], BF16, kind="ExternalInput").ap()
    wvT = nc.dram_tensor("wvT", [128, ND * HEAD_DIM], BF16, kind="ExternalInput").ap()
    woT = nc.dram_tensor("woT", [128, HQ * DIM], BF16, kind="ExternalInput").ap()
    mask4 = nc.dram_tensor("mask4", [128, T], F32, kind="ExternalInput").ap()
    cq = nc.dram_tensor("cq", [128, T], BF16, kind="ExternalInput").ap()
    sq = nc.dram_tensor("sq", [128, T], BF16, kind="ExternalInput").ap()
    ck = nc.dram_tensor("ck", [128, T], BF16, kind="ExternalInput").ap()
    sk = nc.dram_tensor("sk", [128, T], BF16, kind="ExternalInput").ap()
    ident = nc.dram_tensor("ident", [128, 128], BF16, kind="ExternalInput").ap()
    yT = nc.dram_tensor("yT", [DIM, T], BF16, kind="ExternalOutput").ap()

    with tile.TileContext(nc) as tc, ExitStack() as ctx:
        const = ctx.enter_context(tc.tile_pool(name="const", bufs=1))
        wp = ctx.enter_context(tc.tile_pool(name="wp", bufs=6))
        qtp = ctx.enter_context(tc.tile_pool(name="qtp", bufs=4))
        rt = ctx.enter_context(tc.tile_pool(name="rt", bufs=2))
        sm = ctx.enter_context(tc.tile_pool(name="sm", bufs=4))
        yp = ctx.enter_context(tc.tile_pool(name="yp", bufs=4))
        ps = ctx.enter_context(tc.tile_pool(name="ps", bufs=4, space=PSUM))
        psO = ctx.enter_context(tc.tile_pool(name="psO", bufs=4, space=PSUM))

        # ---- PE warm-up: matmuls on a junk tile lift the HAM clock gate and
        # bridge the DMA-arrival window ----
        junk = const.tile([128, T], BF16, tag="junk")
        nc.vector.memset(junk[:], 0.0)
        ps_warm = psO.tile([128, T], F32, tag="psy", name="warm")
        for _ in range(12):
            nc.tensor.matmul(ps_warm[:], junk[:, 0:128], junk[:], start=True, stop=True)

        # ---- small inputs via gpsimd SWDGE (keeps HWDGE rings clear) ----
        ident_sb = const.tile([128, 128], BF16, tag="ident")
        nc.gpsimd.dma_start(ident_sb[:], ident)
        ck_sb = const.tile([128, T], BF16, tag="ck")
        nc.gpsimd.dma_start(ck_sb[:], ck)
        sk_sb = const.tile([128, T], BF16, tag="sk")
        nc.gpsimd.dma_start(sk_sb[:], sk)
        cq_sb = const.tile([128, T], BF16, tag="cq")
        nc.gpsimd.dma_start(cq_sb[:], cq)
        sq_sb = const.tile([128, T], BF16, tag="sq")
        nc.gpsimd.dma_start(sq_sb[:], sq)
        mask_sb = const.tile([128, T], F32, tag="mask4")
        nc.gpsimd.dma_start(mask_sb[:], mask4)

        # ---- big inputs on both HWDGE queues for aggregate bandwidth, but
        # the scalar/ACT engine only issues transfers that finish before its
        # first compute so it can never credit-block the rope/softmax work ----
        x_tiles = [None] * len(XGROUPS)

        def load_x(gi, eng):
            j0, gd = XG_COL[gi]
            xg = const.tile([128, gd * T], BF16, tag=f"x{gi}", name=f"x{gi}")
            eng.dma_start(xg[:], xT[:, j0 * T : (j0 + gd) * T])
            x_tiles[gi] = xg

        wq_tiles = [None] * HQ

        def load_wq(h, eng):
            wqt = wp.tile([128, ND * HEAD_DIM], BF16, tag="w", name=f"wq{h}")
            eng.dma_start(wqt[:], wqT[:, h * DIM : (h + 1) * DIM])
            wq_tiles[h] = wqt

        wk_sb = wp.tile([128, ND * HEAD_DIM], BF16, tag="w", name="wk")
        nc.sync.dma_start(wk_sb[:], wkT)
        wv_sb = wp.tile([128, ND * HEAD_DIM], BF16, tag="w", name="wv")
        nc.scalar.dma_start(wv_sb[:], wvT)
        load_x(0, nc.sync)
        load_x(1, nc.scalar)
        load_x(2, nc.scalar)
        load_wq(0, nc.sync)
        load_x(3, nc.scalar)
        load_x(4, nc.sync)
        load_wq(1, nc.sync)
        load_x(5, nc.scalar)
        load_x(6, nc.sync)
        load_x(7, nc.sync)
        load_x(8, nc.sync)
        load_x(9, nc.sync)
        load_wq(2, nc.sync)
        wo_sb = const.tile([128, HQ * DIM], BF16, tag="wo")
        nc.sync.dma_start(wo_sb[:, 0:DIM], woT[:, 0:DIM])
        load_wq(3, nc.sync)
        for hq in range(1, HQ):
            nc.sync.dma_start(
                wo_sb[:, hq * DIM : (hq + 1) * DIM], woT[:, hq * DIM : (hq + 1) * DIM]
            )

        kT_sb = const.tile([128, T], BF16, tag="kT")
        vT_sb = const.tile([128, T], BF16, tag="vT")
        v_sb = const.tile([128, BSZ * HEAD_DIM], BF16, tag="v")
        oT_sb = const.tile([128, HQ * T], BF16, tag="oT")

        def xslice(j):
            for i, (jj0, gd) in enumerate(XG_COL):
                if jj0 <= j < jj0 + gd:
                    return x_tiles[i][:, (j - jj0) * T : (j - jj0 + 1) * T]
            raise AssertionError(j)

        def rope(dst_ap, pssrc, ctab, stab):
            # evict once to bf16, then all DVE work in SBUF fast mode
            qe = rt.tile([128, T], BF16, tag="qe")
            nc.scalar.copy(qe[:], pssrc[:])
            swp = rt.tile([128, T], BF16, tag="swp")
            nc.vector.tensor_copy(swp[0:64, :], qe[64:128, :])
            nc.vector.tensor_copy(swp[64:128, :], qe[0:64, :])
            prod = rt.tile([128, T], BF16, tag="prod")
            nc.vector.tensor_mul(prod[:], qe[:], ctab[:])
            nc.vector.tensor_mul(swp[:], swp[:], stab[:])
            nc.vector.tensor_add(dst_ap, prod[:], swp[:])

        # ---- K/V staircase riding the x stream: starts on wk + x0 alone,
        # robust to whatever rate the DMA delivers today ----
        ps_k = ps.tile([128, T], F32, tag="ps", name="ps_k")
        ps_v = ps.tile([128, T], F32, tag="ps", name="ps_v")
        for j in range(ND):
            st, sp = (j == 0), (j == ND - 1)
            xr = xslice(j)
            js = slice(j * HEAD_DIM, (j + 1) * HEAD_DIM)
            nc.tensor.matmul(ps_k[:], wk_sb[:, js], xr, start=st, stop=sp)
            nc.tensor.matmul(ps_v[:], wv_sb[:, js], xr, start=st, stop=sp)
            if j in (1, 3, 5, 7):  # x-wait filler keeps the HAM clock gate open
                for _ in range(2):
                    nc.tensor.matmul(
                        ps_warm[:, 0:128], junk[:, 0:128], junk[:, 0:128],
                        start=True, stop=True,
                    )
        rope(kT_sb[:], ps_k[:], ck_sb, sk_sb)
        nc.scalar.copy(vT_sb[:], ps_v[:])
        qts = {}

        def q_sweep(h):
            ps_qh = ps.tile([128, T], F32, tag="ps", name=f"ps_q{h}")
            for j in range(ND):
                st, sp = (j == 0), (j == ND - 1)
                js = slice(j * HEAD_DIM, (j + 1) * HEAD_DIM)
                nc.tensor.matmul(
                    ps_qh[:], wq_tiles[h][:, js], xslice(j), start=st, stop=sp
                )
            qt = qtp.tile([128, T], BF16, tag="qT", name=f"qT{h}")
            rope(qt[:], ps_qh[:], cq_sb, sq_sb)
            return qt

        def att_scores(h):
            ps_s = ps.tile([128, T], F32, tag="ps", name=f"ps_s{h}")
            for b in range(BSZ):
                bs = slice(b * 128, (b + 1) * 128)
                nc.tensor.matmul(
                    ps_s[:, bs], qts[h][:, bs], kT_sb[:, bs], start=True, stop=True
                )
            return ps_s

        # batched softmax over all 4 batches (FD=512 ops); no max-subtract —
        # the fast path gates score sigma < 8 so exp stays inside fp32
        def softmax(h, ps_s):
            s_sb = sm.tile([128, T], F32, tag="s", name=f"s{h}")
            nc.vector.tensor_add(s_sb[:], ps_s[:], mask_sb[:])
            p_sb = sm.tile([128, T], BF16, tag="p", name=f"p{h}")
            nc.scalar.activation(p_sb[:], s_sb[:], ACTF.Exp)
            p3 = p_sb[:].rearrange("p (b k) -> p b k", b=BSZ)
            den = sm.tile([128, BSZ], F32, tag="den", name=f"den{h}")
            nc.vector.reduce_sum(den[:], p3, axis=AX.X)
            rden = sm.tile([128, BSZ], BF16, tag="rden", name=f"rden{h}")
            with nc.allow_low_precision(reason="1/den at 0.4% rel err is fine"):
                nc.vector.reciprocal(rden[:], den[:])
            rb = rden[:].unsqueeze(2).broadcast_to([128, BSZ, 128])
            nc.vector.tensor_mul(p3, p3, rb)
            return p_sb

        def att_ptrans(h, p_sb):
            ps_pt = ps.tile([128, T], BF16, tag="ps", name=f"ps_pt{h}")
            for b in range(BSZ):
                bs = slice(b * 128, (b + 1) * 128)
                nc.tensor.transpose(ps_pt[:, bs], p_sb[:, bs], ident_sb[:])
            pt_sb = sm.tile([128, T], BF16, tag="pt", name=f"pt{h}")
            nc.scalar.copy(pt_sb[:], ps_pt[:])
            return pt_sb

        def att_pv(h, pt_sb):
            ps_o = ps.tile([128, T], F32, tag="ps", name=f"ps_o{h}")
            for b in range(BSZ):
                bs = slice(b * 128, (b + 1) * 128)
                nc.tensor.matmul(
                    ps_o[:, bs], v_sb[:, bs], pt_sb[:, bs], start=True, stop=True
                )
            if h % 2 == 0:
                nc.vector.tensor_copy(oT_sb[:, h * T : (h + 1) * T], ps_o[:])
            else:
                nc.scalar.copy(oT_sb[:, h * T : (h + 1) * T], ps_o[:])

        # q sweeps run back to back so the PE never waits on rope chains;
        # each head's scores/softmax ride under the next sweep
        probs = {}
        qts[0] = q_sweep(0)
        qts[1] = q_sweep(1)
        ps_s0 = att_scores(0)
        probs[0] = softmax(0, ps_s0)  # DVE during q2
        qts[2] = q_sweep(2)
        ps_s1 = att_scores(1)
        probs[1] = softmax(1, ps_s1)  # DVE during q3
        qts[3] = q_sweep(3)
        ps_s2 = att_scores(2)
        probs[2] = softmax(2, ps_s2)
        # V transpose (vT_sb + ident ready long before)
        ps_vt = ps.tile([128, T], BF16, tag="ps", name="ps_vt")
        for b in range(BSZ):
            bs = slice(b * 128, (b + 1) * 128)
            nc.tensor.transpose(ps_vt[:, bs], vT_sb[:, bs], ident_sb[:])
        nc.vector.tensor_copy(v_sb[:], ps_vt[:])

        NE = 4  # early O-proj dtiles interleaved into the attention tail
        ps_y = {}

        def early_o(h, dts):
            for dt in dts:
                if dt not in ps_y:
                    ps_y[dt] = psO.tile([128, T], F32, tag="psy", name=f"ps_y{dt}")
                nc.tensor.matmul(
                    ps_y[dt][:],
                    wo_sb[:, h * DIM + dt * 128 : h * DIM + (dt + 1) * 128],
                    oT_sb[:, h * T : (h + 1) * T],
                    start=(h == 0),
                    stop=False,
                )

        ps_pt0 = att_ptrans(0, probs[0])
        ps_s3 = att_scores(3)
        probs[3] = softmax(3, ps_s3)
        att_pv(0, ps_pt0)
        ps_pt1 = att_ptrans(1, probs[1])
        early_o(0, range(NE))
        att_pv(1, ps_pt1)
        ps_pt2 = att_ptrans(2, probs[2])
        early_o(1, range(NE))
        att_pv(2, ps_pt2)
        ps_pt3 = att_ptrans(3, probs[3])
        early_o(2, range(NE))
        att_pv(3, ps_pt3)

        # ---- output projection over local features (partial sums) ----
        def finish_dtile(dt, ps_ydt, jstart):
            for j in range(jstart, HQ):
                nc.tensor.matmul(
                    ps_ydt[:],
                    wo_sb[:, j * DIM + dt * 128 : j * DIM + (dt + 1) * 128],
                    oT_sb[:, j * T : (j + 1) * T],
                    start=False,
                    stop=(j == HQ - 1),
                )
            y_sb = yp.tile([128, T], BF16, tag="y", name=f"y{dt}")
            if dt % 2 == 0:
                nc.vector.tensor_copy(y_sb[:], ps_ydt[:])
                nc.sync.dma_start(yT[dt * 128 : (dt + 1) * 128, :], y_sb[:])
            else:
                nc.scalar.copy(y_sb[:], ps_ydt[:])
                nc.scalar.dma_start(yT[dt * 128 : (dt + 1) * 128, :], y_sb[:])

        for dt in range(NE):
            finish_dtile(dt, ps_y[dt], 3)

        def finish_last(dt, ps_ydt):
            for j in range(1, HQ):
                nc.tensor.matmul(
                    ps_ydt[:],
                    wo_sb[:, j * DIM + dt * 128 : j * DIM + (dt + 1) * 128],
                    oT_sb[:, j * T : (j + 1) * T],
                    start=False,
                    stop=(j == HQ - 1),
                )
            y_sb = yp.tile([128, T], BF16, tag="y", name=f"y{dt}")
            h = T // 2
            nc.vector.tensor_copy(y_sb[:, :h], ps_ydt[:, :h])
            nc.scalar.copy(y_sb[:, h:], ps_ydt[:, h:])
            nc.sync.dma_start(yT[dt * 128 : (dt + 1) * 128, :h], y_sb[:, :h])
            nc.scalar.dma_start(yT[dt * 128 : (dt + 1) * 128, h:], y_sb[:, h:])
        for dt in range(NE, ND):
            ps_ydt = psO.tile([128, T], F32, tag="psy", name=f"ps_y{dt}")
            nc.tensor.matmul(
                ps_ydt[:],
                wo_sb[:, 0 * DIM + dt * 128 : 0 * DIM + (dt + 1) * 128],
                oT_sb[:, 0:T],
                start=True,
                stop=False,
            )
            if dt == ND - 1:
                finish_last(dt, ps_ydt)
            else:
                finish_dtile(dt, ps_ydt, 1)

    nc.compile()
    return nc


def _build_nc_robust():
    """fp32r q/k path — robust when softmax logits are winner-take-all.

    Kept close to the original structure (max-subtracted softmax)."""
    XD = F32R
    QD = F32
    TD = F32
    nc = bacc.Bacc(
        "TRN2",
        target_bir_lowering=False,
        debug=False,
        enable_asserts=False,
        num_devices=NCORES,
    )
    xT = nc.dram_tensor("xT", [128, ND * T], XD, kind="ExternalInput").ap()
    wqT = nc.dram_tensor("wqT", [128, HQ * ND * HEAD_DIM], XD, kind="ExternalInput").ap()
    wkT = nc.dram_tensor("wkT", [128, ND * HEAD_DIM], XD, kind="ExternalInput").ap()
    wvT = nc.dram_tensor("wvT", [128, ND * HEAD_DIM], XD, kind="ExternalInput").ap()
    woT = nc.dram_tensor("woT", [128, HQ * DIM], BF16, kind="ExternalInput").ap()
    mask1 = nc.dram_tensor("mask1", [128, 128], F32, kind="ExternalInput").ap()
    cq = nc.dram_tensor("cq", [128, T], TD, kind="ExternalInput").ap()
    sq = nc.dram_tensor("sq", [128, T], TD, kind="ExternalInput").ap()
    ck = nc.dram_tensor("ck", [128, T], TD, kind="ExternalInput").ap()
    sk = nc.dram_tensor("sk", [128, T], TD, kind="ExternalInput").ap()
    ident = nc.dram_tensor("ident", [128, 128], BF16, kind="ExternalInput").ap()
    yT = nc.dram_tensor("yT", [DIM, T], BF16, kind="ExternalOutput").ap()

    with tile.TileContext(nc) as tc, ExitStack() as ctx:
        const = ctx.enter_context(tc.tile_pool(name="const", bufs=1))
        wp = ctx.enter_context(tc.tile_pool(name="wp", bufs=4))
        qtp = ctx.enter_context(tc.tile_pool(name="qtp", bufs=4))
        rt = ctx.enter_context(tc.tile_pool(name="rt", bufs=1))
        sm = ctx.enter_context(tc.tile_pool(name="sm", bufs=2))
        yp = ctx.enter_context(tc.tile_pool(name="yp", bufs=2))
        ps = ctx.enter_context(tc.tile_pool(name="ps", bufs=7, space=PSUM))
        wps = ctx.enter_context(tc.tile_pool(name="wps", bufs=1, space=PSUM))

        warm_w = const.tile([128, 128], BF16, tag="warm_w")
        nc.vector.memset(warm_w[:], 0.0)
        warm_x = const.tile([128, T], BF16, tag="warm_x")
        nc.vector.memset(warm_x[:], 0.0)
        ps_warm = wps.tile([128, T], F32, tag="wps")
        for _ in range(10):
            nc.tensor.matmul(ps_warm[:], warm_w[:], warm_x[:], start=True, stop=True)

        wk_sb = wp.tile([128, ND * HEAD_DIM], XD, tag="w", name="wk")
        nc.sync.dma_start(wk_sb[:], wkT)
        wv_sb = wp.tile([128, ND * HEAD_DIM], XD, tag="w", name="wv")
        nc.scalar.dma_start(wv_sb[:], wvT)

        x_tiles = [None] * len(XGROUPS)

        def load_x(gi, eng):
            j0, gd = XG_COL[gi]
            xg = const.tile([128, gd * T], XD, tag=f"x{gi}", name=f"x{gi}")
            eng.dma_start(xg[:], xT[:, j0 * T : (j0 + gd) * T])
            x_tiles[gi] = xg

        wq_tiles = [None] * HQ

        def load_wq(h, eng):
            wqt = wp.tile([128, ND * HEAD_DIM], XD, tag="w", name=f"wq{h}")
            eng.dma_start(wqt[:], wqT[:, h * DIM : (h + 1) * DIM])
            wq_tiles[h] = wqt

        load_x(0, nc.sync)
        load_wq(0, nc.scalar)
        load_wq(1, nc.sync)
        load_x(1, nc.scalar)
        load_wq(2, nc.sync)
        load_x(2, nc.scalar)
        load_wq(3, nc.sync)
        ident_sb = const.tile([128, 128], BF16, tag="ident")
        nc.scalar.dma_start(ident_sb[:], ident)
        ck_sb = const.tile([128, T], TD, tag="ck")
        nc.scalar.dma_start(ck_sb[:], ck)
        sk_sb = const.tile([128, T], TD, tag="sk")
        nc.scalar.dma_start(sk_sb[:], sk)
        cq_sb = const.tile([128, T], TD, tag="cq")
        nc.scalar.dma_start(cq_sb[:], cq)
        sq_sb = const.tile([128, T], TD, tag="sq")
        nc.scalar.dma_start(sq_sb[:], sq)
        mask_sb = const.tile([128, 128], F32, tag="mask")
        nc.scalar.dma_start(mask_sb[:], mask1)
        for gi in range(3, len(XGROUPS)):
            load_x(gi, nc.scalar if gi % 2 == 0 else nc.sync)
        wo_sb = const.tile([128, HQ * DIM], BF16, tag="wo")
        nc.sync.dma_start(wo_sb[:, : 2 * DIM], woT[:, : 2 * DIM])
        nc.scalar.dma_start(wo_sb[:, 2 * DIM :], woT[:, 2 * DIM :])

        kT_sb = const.tile([128, T], QD, tag="kT")
        vT_sb = const.tile([128, T], BF16, tag="vT")
        v_sb = const.tile([128, BSZ * HEAD_DIM], BF16, tag="v")
        oT_sb = const.tile([128, HQ * T], BF16, tag="oT")

        def xslice(j):
            for i, (jj0, gd) in enumerate(XG_COL):
                if jj0 <= j < jj0 + gd:
                    return x_tiles[i][:, (j - jj0) * T : (j - jj0 + 1) * T]
            raise AssertionError(j)

        def rope(dst_ap, pssrc, ctab, stab):
            swp = rt.tile([128, T], F32, tag="swp")
            nc.scalar.copy(swp[0:64, :], pssrc[64:128, :])
            nc.scalar.copy(swp[64:128, :], pssrc[0:64, :])
            prod = rt.tile([128, T], F32, tag="prod")
            nc.vector.tensor_mul(prod[:], pssrc[:], ctab)
            nc.vector.tensor_mul(swp[:], swp[:], stab)
            nc.vector.tensor_add(dst_ap, prod[:], swp[:])

        ps_k = ps.tile([128, T], F32, tag="ps")
        ps_v = ps.tile([128, T], F32, tag="ps")
        ps_q = [None] * HQ
        NSW = 2
        for h in range(NSW):
            ps_q[h] = ps.tile([128, T], F32, tag="ps", name=f"ps_q{h}")
        for j in range(ND):
            st, sp = (j == 0), (j == ND - 1)
            xr = xslice(j)
            js = slice(j * HEAD_DIM, (j + 1) * HEAD_DIM)
            nc.tensor.matmul(ps_k[:], wk_sb[:, js], xr, start=st, stop=sp)
            nc.tensor.matmul(ps_v[:], wv_sb[:, js], xr, start=st, stop=sp)
            for h in range(NSW):
                nc.tensor.matmul(ps_q[h][:], wq_tiles[h][:, js], xr, start=st, stop=sp)

        rope(kT_sb[:], ps_k[:], ck_sb[:], sk_sb[:])
        qts = {}
        for h in range(NSW):
            qts[h] = qtp.tile([128, T], QD, tag="qT", name=f"qT{h}")
            rope(qts[h][:], ps_q[h][:], cq_sb[:], sq_sb[:])

        def q_sweep(h):
            ps_qh = ps.tile([128, T], F32, tag="ps", name=f"ps_q{h}")
            for j in range(ND):
                st, sp = (j == 0), (j == ND - 1)
                js = slice(j * HEAD_DIM, (j + 1) * HEAD_DIM)
                nc.tensor.matmul(
                    ps_qh[:], wq_tiles[h][:, js], xslice(j), start=st, stop=sp
                )
            qt = qtp.tile([128, T], QD, tag="qT", name=f"qT{h}")
            rope(qt[:], ps_qh[:], cq_sb[:], sq_sb[:])
            return qt

        def keep_warm(n=2):
            for _ in range(n):
                nc.tensor.matmul(
                    ps_warm[:], warm_w[:], warm_x[:], start=True, stop=True
                )

        def att_scores(h, qt):
            ps_s = ps.tile([128, T], F32, tag="ps", name=f"ps_s{h}")
            for b in range(BSZ):
                bs = slice(b * 128, (b + 1) * 128)
                nc.tensor.matmul(
                    ps_s[:, bs], qt[:, bs], kT_sb[:, bs], start=True, stop=True
                )
            s_sb = sm.tile([128, T], F32, tag="s", name=f"s{h}")
            nmx = sm.tile([128, BSZ], F32, tag="nmx", name=f"nmx{h}")
            den = sm.tile([128, BSZ], F32, tag="den", name=f"den{h}")
            rden = sm.tile([128, BSZ], F32, tag="rden", name=f"rden{h}")
            p_sb = sm.tile([128, T], BF16, tag="p", name=f"p{h}")
            for b in range(BSZ):
                bs = slice(b * 128, (b + 1) * 128)
                nc.vector.tensor_add(s_sb[:, bs], ps_s[:, bs], mask_sb[:])
                nc.vector.reduce_max(
                    nmx[:, b : b + 1], s_sb[:, bs], axis=AX.X, negate=True
                )
                nc.scalar.activation(
                    p_sb[:, bs],
                    s_sb[:, bs],
                    ACTF.Exp,
                    bias=nmx[:, b : b + 1],
                    accum_out=den[:, b : b + 1],
                )
            nc.vector.reciprocal(rden[:], den[:])
            for b in range(BSZ):
                bs = slice(b * 128, (b + 1) * 128)
                nc.vector.tensor_scalar_mul(p_sb[:, bs], p_sb[:, bs], rden[:, b : b + 1])
            return p_sb

        def att_ptrans(h, p_sb):
            ps_pt = ps.tile([128, T], BF16, tag="ps", name=f"ps_pt{h}")
            for b in range(BSZ):
                bs = slice(b * 128, (b + 1) * 128)
                nc.tensor.transpose(ps_pt[:, bs], p_sb[:, bs], ident_sb[:])
            pt_sb = sm.tile([128, T], BF16, tag="pt", name=f"pt{h}")
            nc.scalar.copy(pt_sb[:], ps_pt[:])
            return pt_sb

        def att_pv(h, pt_sb):
            ps_o = ps.tile([128, T], F32, tag="ps", name=f"ps_o{h}")
            for b in range(BSZ):
                bs = slice(b * 128, (b + 1) * 128)
                nc.tensor.matmul(
                    ps_o[:, bs], v_sb[:, bs], pt_sb[:, bs], start=True, stop=True
                )
            if h % 2 == 0:
                nc.vector.tensor_copy(oT_sb[:, h * T : (h + 1) * T], ps_o[:])
            else:
                nc.scalar.copy(oT_sb[:, h * T : (h + 1) * T], ps_o[:])

        probs = {}
        qts[2] = q_sweep(2)
        nc.scalar.copy(vT_sb[:], ps_v[:])
        for b in range(BSZ):
            bs = slice(b * 128, (b + 1) * 128)
            ps_t = ps.tile([128, T], BF16, tag="ps")
            nc.tensor.transpose(ps_t[:, 0:128], vT_sb[:, bs], ident_sb[:])
            nc.vector.tensor_copy(v_sb[:, bs], ps_t[:, 0:128])
        probs[0] = att_scores(0, qts[0])
        probs[1] = att_scores(1, qts[1])
        qts[3] = q_sweep(3)
        att_pv(0, probs[0])
        probs[2] = att_scores(2, qts[2])
        att_pv(1, probs[1])
        keep_warm(2)
        probs[3] = att_scores(3, qts[3])
        att_pv(2, probs[2])
        keep_warm(2)
        att_pv(3, probs[3])

        for dt in range(ND):
            ps_y = ps.tile([128, T], F32, tag="ps", name=f"ps_y{dt}")
            for j in range(HQ):
                nc.tensor.matmul(
                    ps_y[:],
                    wo_sb[:, j * DIM + dt * 128 : j * DIM + (dt + 1) * 128],
                    oT_sb[:, j * T : (j + 1) * T],
                    start=(j == 0),
                    stop=(j == HQ - 1),
                )
            y_sb = yp.tile([128, T], BF16, tag="y", name=f"y{dt}")
            if dt % 2 == 0:
                nc.vector.tensor_copy(y_sb[:], ps_y[:])
                nc.sync.dma_start(yT[dt * 128 : (dt + 1) * 128, :], y_sb[:])
            else:
                nc.scalar.copy(y_sb[:], ps_y[:])
                nc.scalar.dma_start(yT[dt * 128 : (dt + 1) * 128, :], y_sb[:])

    nc.compile()
    return nc


def get_nc(fast: bool):
    key = "nc_fast" if fast else "nc_robust"
    if key not in _STATE:
        _STATE[key] = _build_nc_fast() if fast else _build_nc_robust()
    return _STATE[key]


def _prep_in_maps(x, wq, wk, wv, wo, freqs_cos, freqs_sin, mask, fast):
    f32 = np.float32
    bf16 = ml_dtypes.bfloat16
    xd = bf16 if fast else f32
    x = np.asarray(x, f32)
    wq = np.asarray(wq, f32)
    wk = np.asarray(wk, f32)
    wv = np.asarray(wv, f32)
    wo = np.asarray(wo, f32)
    fc = np.asarray(freqs_cos, f32)
    fs = np.asarray(freqs_sin, f32)
    mask = np.asarray(mask, f32)

    # even features first, then odd: (2i, 2i+1) pairs -> (i, i+64)
    perm = np.concatenate([np.arange(0, HEAD_DIM, 2), np.arange(1, HEAD_DIM, 2)])
    wqp = wq.reshape(N_HEADS, HEAD_DIM, DIM)[:, perm, :].reshape(DIM, DIM)
    wkp = wk.reshape(N_KV_HEADS, HEAD_DIM, DIM)[:, perm, :].reshape(
        N_KV_HEADS * HEAD_DIM, DIM
    )

    def sw_x(xmat):  # [T, DIM] -> [128, ND*T]
        return np.ascontiguousarray(
            xmat.T.reshape(ND, 128, T).transpose(1, 0, 2).reshape(128, ND * T)
        )

    def sw_w(wmat):  # [E(128), DIM] -> [128, ND*E]
        E = wmat.shape[0]
        return np.ascontiguousarray(
            wmat.T.reshape(ND, 128, E).transpose(1, 0, 2).reshape(128, ND * E)
        )

    xT = sw_x(x.reshape(T, DIM)).astype(xd)
    C0 = np.vstack([fc.T, fc.T])  # [128, 128]: row p -> cos[t, p % 64]
    S0 = np.vstack([-fs.T, fs.T])
    td = bf16 if fast else f32
    cq = np.ascontiguousarray(np.tile(C0 * SCALE, (1, BSZ))).astype(td)
    sq = np.ascontiguousarray(np.tile(S0 * SCALE, (1, BSZ))).astype(td)
    ck = np.ascontiguousarray(np.tile(C0, (1, BSZ))).astype(td)
    sk = np.ascontiguousarray(np.tile(S0, (1, BSZ))).astype(td)
    ident = np.eye(128, dtype=bf16)

    in_maps = []
    for c in range(NCORES):
        qrows = slice(c * EQ, (c + 1) * EQ)
        krows = slice(c * HEAD_DIM, (c + 1) * HEAD_DIM)
        wq_heads = [
            sw_w(wqp[c * EQ + h * HEAD_DIM : c * EQ + (h + 1) * HEAD_DIM, :])
            for h in range(HQ)
        ]
        wo_sw = np.ascontiguousarray(
            wo[:, qrows].T.reshape(HQ, 128, DIM).transpose(1, 0, 2).reshape(128, HQ * DIM)
        ).astype(bf16)
        m = {
            "xT": xT,
            "wqT": np.ascontiguousarray(np.concatenate(wq_heads, axis=1)).astype(xd),
            "wkT": sw_w(wkp[krows, :]).astype(xd),
            "wvT": sw_w(wv[krows, :]).astype(xd),
            "woT": wo_sw,
            "cq": cq,
            "sq": sq,
            "ck": ck,
            "sk": sk,
            "ident": ident,
        }
        if fast:
            m["mask4"] = np.ascontiguousarray(np.tile(mask[0, 0], (1, BSZ))).astype(f32)
        else:
            m["mask1"] = np.ascontiguousarray(mask[0, 0])
        in_maps.append(m)
    return in_maps


def _pick_fast(x, wq):
    """bf16 q/k only when softmax logits are smooth (score sigma small)."""
    sx = float(np.asarray(x, np.float32).std())
    sw = float(np.asarray(wq, np.float32).std())
    sigma = sx * sw * math.sqrt(DIM * HEAD_DIM) * SCALE
    return sigma < 8.0


def kernel(
    x,
    wq,
    wk,
    wv,
    wo,
    cache_k,
    cache_v,
    freqs_cos,
    freqs_sin,
    mask,
    start_pos,
    *,
    trace=False,
    trace_kwargs=None,
):
    global LAST_RESULT
    sp = int(np.asarray(start_pos))
    assert sp == 0, f"kernel specialized for start_pos=0, got {sp}"

    fast = _pick_fast(x, wq)
    in_maps = _prep_in_maps(x, wq, wk, wv, wo, freqs_cos, freqs_sin, mask, fast)
    nc = get_nc(fast)
    res = run_bass_kernel_spmd(
        nc,
        in_maps,
        core_ids=list(range(NCORES)),
        trace=trace,
        **(trace_kwargs or {}),
    )
    LAST_RESULT = res
    acc = np.zeros((DIM, T), np.float32)
    for c in range(NCORES):
        acc += res.results[c]["yT"].astype(np.float32)
    return np.ascontiguousarray(acc.T).reshape(BSZ, SEQLEN, DIM)


# revision 13
# speedup vs baseline: 1.1390x; 1.1390x over previous
"""Tensor-parallel GQA attention prefill for 8 TRN2 NeuronCores.

Sharding: each core owns 4 query heads + 1 kv head (column-shard of
wq/wk/wv by head) and a 512-row slice of wo's input dim (row-shard).
Each core computes a partial output projection over its local heads;
the host sums the 8 partials (the all-reduce) and transposes back.

Fast path (bf16) structure, tuned from perfetto/NTFF traces:
  - warmup matmuls on a junk tile (no gpsimd memset) to lift the HAM
    clock gate while DMAs land
  - K+V sweep first (needs only wk+wv), riding the x stream; Q-head
    sweeps follow from SBUF-resident x. Weights for later sweeps are
    queued behind x; wq2/wq3/wo go via the gpsimd SWDGE queue so the
    two HWDGE rings carry only the critical stream and the scalar
    engine is not credit-blocked when rope/softmax work starts.
  - scores computed TRANSPOSED (keys on partitions): softmax denom is
    a ones-vector matmul (partition reduction on PE), no P-transpose,
    normalization folded into the oT eviction via a PE-broadcast of
    1/den. No max-subtraction (fast path gates sigma < 8, exp stays
    comfortably inside fp32).
  - O-projection starts early on heads 0/1 for the first psum banks
    while the last head's softmax drains.
"""

import math
from contextlib import ExitStack

import ml_dtypes
import numpy as np

import concourse.bass as bass
import concourse.tile as tile
from concourse import bacc, mybir
from concourse.bass_utils import run_bass_kernel_spmd

DIM = 4096
N_HEADS = 32
HEAD_DIM = 128
N_KV_HEADS = 8
BSZ = 4
SEQLEN = 128
T = BSZ * SEQLEN  # 512 tokens
NCORES = 8
HQ = N_HEADS // NCORES  # 4 query heads per core
EQ = HQ * HEAD_DIM  # 512 local q features
ND = DIM // 128  # 32 contraction tiles
SCALE = 1.0 / math.sqrt(HEAD_DIM)

F32 = mybir.dt.float32
F32R = mybir.dt.float32r
BF16 = mybir.dt.bfloat16
AX = mybir.AxisListType
ACTF = mybir.ActivationFunctionType
PSUM = bass.MemorySpace.PSUM

_STATE: dict = {}
LAST_RESULT = None


def _install_ntff_hook():
    """Register the axon NTFF profile hook if the image lacks antenv.axon_hooks."""
    import os
    import sys
    import types

    try:
        import antenv.axon_hooks  # noqa: F401

        return
    except ImportError:
        pass
    try:
        import antenv
        from trn_agent_boot.trn_boot import _ntff_profile_via_ctypes

        mod = types.ModuleType("antenv.axon_hooks")
        holder = {"hook": None}
        mod.set_axon_ntff_profile_hook = lambda h: holder.__setitem__("hook", h)
        mod.get_axon_ntff_profile_hook = lambda: holder["hook"]
        sys.modules["antenv.axon_hooks"] = mod
        antenv.axon_hooks = mod
        so = "/opt/axon/libaxon_pjrt.so"
        if os.path.exists(so):
            hook = _ntff_profile_via_ctypes(so)
            if hook is not None:
                mod.set_axon_ntff_profile_hook(hook)
    except Exception:
        pass


_install_ntff_hook()

XGROUPS = [2, 2, 2, 2, 4, 4, 4, 4, 4, 4]
XG_COL = []
_j0 = 0
for _gd in XGROUPS:
    XG_COL.append((_j0, _gd))
    _j0 += _gd


def _build_nc_fast():
    nc = bacc.Bacc(
        "TRN2",
        target_bir_lowering=False,
        debug=False,
        enable_asserts=False,
        num_devices=NCORES,
    )
    xT = nc.dram_tensor("xT", [128, ND * T], BF16, kind="ExternalInput").ap()
    wqT = nc.dram_tensor("wqT", [128, HQ * ND * HEAD_DIM], BF16, kind="ExternalInput").ap()
    wkT = nc.dram_tensor("wkT", [128, ND * HEAD_DIM], BF16, kind="ExternalInput").ap()
    wvT = nc.dram_tensor("wvT", [128, ND * HEAD_DIM], BF16, kind="ExternalInput").ap()
    woT = nc.dram_tensor("woT", [128, HQ * DIM], BF16, kind="ExternalInput").ap()
    mask4 = nc.dram_tensor("mask4", [128, T], F32, kind="ExternalInput").ap()
    cq = nc.dram_tensor("cq", [128, T], BF16, kind="ExternalInput").ap()
    sq = nc.dram_tensor("sq", [128, T], BF16, kind="ExternalInput").ap()
    ck = nc.dram_tensor("ck", [128, T], BF16, kind="ExternalInput").ap()
    sk = nc.dram_tensor("sk", [128, T], BF16, kind="ExternalInput").ap()
    ident = nc.dram_tensor("ident", [128, 128], BF16, kind="ExternalInput").ap()
    yT = nc.dram_tensor("yT", [DIM, T], BF16, kind="ExternalOutput").ap()

    with tile.TileContext(nc) as tc, ExitStack() as ctx:
        const = ctx.enter_context(tc.tile_pool(name="const", bufs=1))
        wp = ctx.enter_context(tc.tile_pool(name="wp", bufs=6))
        qtp = ctx.enter_context(tc.tile_pool(name="qtp", bufs=4))
        rt = ctx.enter_context(tc.tile_pool(name="rt", bufs=2))
        sm = ctx.enter_context(tc.tile_pool(name="sm", bufs=4))
        yp = ctx.enter_context(tc.tile_pool(name="yp", bufs=4))
        ps = ctx.enter_context(tc.tile_pool(name="ps", bufs=4, space=PSUM))
        psO = ctx.enter_context(tc.tile_pool(name="psO", bufs=4, space=PSUM))

        # ---- PE warm-up: matmuls on a junk tile lift the HAM clock gate and
        # bridge the DMA-arrival window ----
        junk = const.tile([128, T], BF16, tag="junk")
        nc.vector.memset(junk[:], 0.0)
        ps_warm = psO.tile([128, T], F32, tag="psy", name="warm")
        for _ in range(12):
            nc.tensor.matmul(ps_warm[:], junk[:, 0:128], junk[:], start=True, stop=True)

        # ---- small inputs via gpsimd SWDGE (keeps HWDGE rings clear) ----
        ident_sb = const.tile([128, 128], BF16, tag="ident")
        nc.gpsimd.dma_start(ident_sb[:], ident)
        ck_sb = const.tile([128, T], BF16, tag="ck")
        nc.gpsimd.dma_start(ck_sb[:], ck)
        sk_sb = const.tile([128, T], BF16, tag="sk")
        nc.gpsimd.dma_start(sk_sb[:], sk)
        cq_sb = const.tile([128, T], BF16, tag="cq")
        nc.gpsimd.dma_start(cq_sb[:], cq)
        sq_sb = const.tile([128, T], BF16, tag="sq")
        nc.gpsimd.dma_start(sq_sb[:], sq)
        mask_sb = const.tile([128, T], F32, tag="mask4")
        nc.gpsimd.dma_start(mask_sb[:], mask4)

        # ---- big inputs on both HWDGE queues for aggregate bandwidth, but
        # the scalar/ACT engine only issues transfers that finish before its
        # first compute so it can never credit-block the rope/softmax work ----
        x_tiles = [None] * len(XGROUPS)

        def load_x(gi, eng):
            j0, gd = XG_COL[gi]
            xg = const.tile([128, gd * T], BF16, tag=f"x{gi}", name=f"x{gi}")
            eng.dma_start(xg[:], xT[:, j0 * T : (j0 + gd) * T])
            x_tiles[gi] = xg

        wq_tiles = [None] * HQ

        def load_wq(h, eng):
            wqt = wp.tile([128, ND * HEAD_DIM], BF16, tag="w", name=f"wq{h}")
            eng.dma_start(wqt[:], wqT[:, h * DIM : (h + 1) * DIM])
            wq_tiles[h] = wqt

        wk_sb = wp.tile([128, ND * HEAD_DIM], BF16, tag="w", name="wk")
        nc.sync.dma_start(wk_sb[:], wkT)
        wv_sb = wp.tile([128, ND * HEAD_DIM], BF16, tag="w", name="wv")
        nc.scalar.dma_start(wv_sb[:], wvT)
        load_x(0, nc.sync)
        load_x(1, nc.scalar)
        load_x(2, nc.scalar)
        load_wq(0, nc.sync)
        load_x(3, nc.scalar)
        load_x(4, nc.sync)
        load_wq(1, nc.sync)
        load_x(5, nc.scalar)
        load_x(6, nc.sync)
        load_x(7, nc.sync)
        load_x(8, nc.sync)
        load_x(9, nc.sync)
        load_wq(2, nc.sync)
        wo_sb = const.tile([128, HQ * DIM], BF16, tag="wo")
        nc.sync.dma_start(wo_sb[:, 0:DIM], woT[:, 0:DIM])
        load_wq(3, nc.sync)
        for hq in range(1, HQ):
            nc.sync.dma_start(
                wo_sb[:, hq * DIM : (hq + 1) * DIM], woT[:, hq * DIM : (hq + 1) * DIM]
            )

        kT_sb = const.tile([128, T], BF16, tag="kT")
        vT_sb = const.tile([128, T], BF16, tag="vT")
        v_sb = const.tile([128, BSZ * HEAD_DIM], BF16, tag="v")
        oT_sb = const.tile([128, HQ * T], BF16, tag="oT")

        def xslice(j):
            for i, (jj0, gd) in enumerate(XG_COL):
                if jj0 <= j < jj0 + gd:
                    return x_tiles[i][:, (j - jj0) * T : (j - jj0 + 1) * T]
            raise AssertionError(j)

        def rope(dst_ap, pssrc, ctab, stab):
            # evict once to bf16, then all DVE work in SBUF fast mode
            qe = rt.tile([128, T], BF16, tag="qe")
            nc.scalar.copy(qe[:], pssrc[:])
            swp = rt.tile([128, T], BF16, tag="swp")
            nc.vector.tensor_copy(swp[0:64, :], qe[64:128, :])
            nc.vector.tensor_copy(swp[64:128, :], qe[0:64, :])
            prod = rt.tile([128, T], BF16, tag="prod")
            nc.vector.tensor_mul(prod[:], qe[:], ctab[:])
            nc.vector.tensor_mul(swp[:], swp[:], stab[:])
            nc.vector.tensor_add(dst_ap, prod[:], swp[:])

        # ---- K/V staircase riding the x stream: starts on wk + x0 alone,
        # robust to whatever rate the DMA delivers today ----
        ps_k = ps.tile([128, T], F32, tag="ps", name="ps_k")
        ps_v = ps.tile([128, T], F32, tag="ps", name="ps_v")
        for j in range(ND):
            st, sp = (j == 0), (j == ND - 1)
            xr = xslice(j)
            js = slice(j * HEAD_DIM, (j + 1) * HEAD_DIM)
            nc.tensor.matmul(ps_k[:], wk_sb[:, js], xr, start=st, stop=sp)
            nc.tensor.matmul(ps_v[:], wv_sb[:, js], xr, start=st, stop=sp)
            if j in (1, 3, 5, 7):  # x-wait filler keeps the HAM clock gate open
                for _ in range(2):
                    nc.tensor.matmul(
                        ps_warm[:, 0:128], junk[:, 0:128], junk[:, 0:128],
                        start=True, stop=True,
                    )
        rope(kT_sb[:], ps_k[:], ck_sb, sk_sb)
        nc.scalar.copy(vT_sb[:], ps_v[:])
        qts = {}

        def q_sweep(h):
            ps_qh = ps.tile([128, T], F32, tag="ps", name=f"ps_q{h}")
            for j in range(ND):
                st, sp = (j == 0), (j == ND - 1)
                js = slice(j * HEAD_DIM, (j + 1) * HEAD_DIM)
                nc.tensor.matmul(
                    ps_qh[:], wq_tiles[h][:, js], xslice(j), start=st, stop=sp
                )
            qt = qtp.tile([128, T], BF16, tag="qT", name=f"qT{h}")
            rope(qt[:], ps_qh[:], cq_sb, sq_sb)
            return qt

        def att_scores(h):
            ps_s = ps.tile([128, T], F32, tag="ps", name=f"ps_s{h}")
            for b in range(BSZ):
                bs = slice(b * 128, (b + 1) * 128)
                nc.tensor.matmul(
                    ps_s[:, bs], qts[h][:, bs], kT_sb[:, bs], start=True, stop=True
                )
            return ps_s

        # batched softmax over all 4 batches (FD=512 ops); no max-subtract —
        # the fast path gates score sigma < 8 so exp stays inside fp32
        def softmax(h, ps_s):
            s_sb = sm.tile([128, T], F32, tag="s", name=f"s{h}")
            nc.vector.tensor_add(s_sb[:], ps_s[:], mask_sb[:])
            p_sb = sm.tile([128, T], BF16, tag="p", name=f"p{h}")
            nc.scalar.activation(p_sb[:], s_sb[:], ACTF.Exp)
            p3 = p_sb[:].rearrange("p (b k) -> p b k", b=BSZ)
            den = sm.tile([128, BSZ], F32, tag="den", name=f"den{h}")
            nc.vector.reduce_sum(den[:], p3, axis=AX.X)
            rden = sm.tile([128, BSZ], BF16, tag="rden", name=f"rden{h}")
            with nc.allow_low_precision(reason="1/den at 0.4% rel err is fine"):
                nc.vector.reciprocal(rden[:], den[:])
            rb = rden[:].unsqueeze(2).broadcast_to([128, BSZ, 128])
            nc.vector.tensor_mul(p3, p3, rb)
            return p_sb

        def att_ptrans(h, p_sb):
            ps_pt = ps.tile([128, T], BF16, tag="ps", name=f"ps_pt{h}")
            for b in range(BSZ):
                bs = slice(b * 128, (b + 1) * 128)
                nc.tensor.transpose(ps_pt[:, bs], p_sb[:, bs], ident_sb[:])
            pt_sb = sm.tile([128, T], BF16, tag="pt", name=f"pt{h}")
            nc.scalar.copy(pt_sb[:], ps_pt[:])
            return pt_sb

        def att_pv(h, pt_sb):
            ps_o = ps.tile([128, T], F32, tag="ps", name=f"ps_o{h}")
            for b in range(BSZ):
                bs = slice(b * 128, (b + 1) * 128)
                nc.tensor.matmul(
                    ps_o[:, bs], v_sb[:, bs], pt_sb[:, bs], start=True, stop=True
                )
            if h % 2 == 0:
                nc.vector.tensor_copy(oT_sb[:, h * T : (h + 1) * T], ps_o[:])
            else:
                nc.scalar.copy(oT_sb[:, h * T : (h + 1) * T], ps_o[:])

        # q sweeps run back to back so the PE never waits on rope chains;
        # each head's scores/softmax/P-transpose ride under the next sweep
        # and the per-head PV + early O-proj chunks fill the softmax tail
        probs = {}
        qts[0] = q_sweep(0)
        qts[1] = q_sweep(1)
        ps_s0 = att_scores(0)
        probs[0] = softmax(0, ps_s0)  # DVE during q2
        # V transpose (vT_sb + ident ready long before); copy rides under q2
        ps_vt = ps.tile([128, T], BF16, tag="ps", name="ps_vt")
        for b in range(BSZ):
            bs = slice(b * 128, (b + 1) * 128)
            nc.tensor.transpose(ps_vt[:, bs], vT_sb[:, bs], ident_sb[:])
        nc.vector.tensor_copy(v_sb[:], ps_vt[:])
        qts[2] = q_sweep(2)
        ps_s1 = att_scores(1)
        probs[1] = softmax(1, ps_s1)  # DVE during q3
        pt0 = att_ptrans(0, probs[0])  # pt copy rides under q3
        qts[3] = q_sweep(3)
        ps_s2 = att_scores(2)
        probs[2] = softmax(2, ps_s2)
        att_pv(0, pt0)

        NE = 4  # early O-proj dtiles interleaved into the attention tail
        ps_y = {}

        def early_o(h, dts):
            for dt in dts:
                if dt not in ps_y:
                    ps_y[dt] = psO.tile([128, T], F32, tag="psy", name=f"ps_y{dt}")
                nc.tensor.matmul(
                    ps_y[dt][:],
                    wo_sb[:, h * DIM + dt * 128 : h * DIM + (dt + 1) * 128],
                    oT_sb[:, h * T : (h + 1) * T],
                    start=(h == 0),
                    stop=False,
                )

        pt1 = att_ptrans(1, probs[1])
        early_o(0, range(NE))
        ps_s3 = att_scores(3)
        probs[3] = softmax(3, ps_s3)
        att_pv(1, pt1)
        pt2 = att_ptrans(2, probs[2])
        early_o(1, range(NE))
        att_pv(2, pt2)
        pt3 = att_ptrans(3, probs[3])
        early_o(2, range(NE))
        att_pv(3, pt3)

        # ---- output projection over local features (partial sums) ----
        def finish_dtile(dt, ps_ydt, jstart):
            for j in range(jstart, HQ):
                nc.tensor.matmul(
                    ps_ydt[:],
                    wo_sb[:, j * DIM + dt * 128 : j * DIM + (dt + 1) * 128],
                    oT_sb[:, j * T : (j + 1) * T],
                    start=False,
                    stop=(j == HQ - 1),
                )
            y_sb = yp.tile([128, T], BF16, tag="y", name=f"y{dt}")
            if dt % 2 == 0:
                nc.vector.tensor_copy(y_sb[:], ps_ydt[:])
                nc.sync.dma_start(yT[dt * 128 : (dt + 1) * 128, :], y_sb[:])
            else:
                nc.scalar.copy(y_sb[:], ps_ydt[:])
                nc.scalar.dma_start(yT[dt * 128 : (dt + 1) * 128, :], y_sb[:])

        for dt in range(NE):
            finish_dtile(dt, ps_y[dt], 3)

        def finish_last(dt, ps_ydt):
            for j in range(1, HQ):
                nc.tensor.matmul(
                    ps_ydt[:],
                    wo_sb[:, j * DIM + dt * 128 : j * DIM + (dt + 1) * 128],
                    oT_sb[:, j * T : (j + 1) * T],
                    start=False,
                    stop=(j == HQ - 1),
                )
            y_sb = yp.tile([128, T], BF16, tag="y", name=f"y{dt}")
            h = T // 2
            nc.vector.tensor_copy(y_sb[:, :h], ps_ydt[:, :h])
            nc.scalar.copy(y_sb[:, h:], ps_ydt[:, h:])
            nc.sync.dma_start(yT[dt * 128 : (dt + 1) * 128, :h], y_sb[:, :h])
            nc.scalar.dma_start(yT[dt * 128 : (dt + 1) * 128, h:], y_sb[:, h:])
        for dt in range(NE, ND):
            ps_ydt = psO.tile([128, T], F32, tag="psy", name=f"ps_y{dt}")
            nc.tensor.matmul(
                ps_ydt[:],
                wo_sb[:, 0 * DIM + dt * 128 : 0 * DIM + (dt + 1) * 128],
                oT_sb[:, 0:T],
                start=True,
                stop=False,
            )
            if dt == ND - 1:
                finish_last(dt, ps_ydt)
            else:
                finish_dtile(dt, ps_ydt, 1)

    nc.compile()
    return nc


def _build_nc_robust():
    """fp32r q/k path — robust when softmax logits are winner-take-all.

    Kept close to the original structure (max-subtracted softmax)."""
    XD = F32R
    QD = F32
    TD = F32
    nc = bacc.Bacc(
        "TRN2",
        target_bir_lowering=False,
        debug=False,
        enable_asserts=False,
        num_devices=NCORES,
    )
    xT = nc.dram_tensor("xT", [128, ND * T], XD, kind="ExternalInput").ap()
    wqT = nc.dram_tensor("wqT", [128, HQ * ND * HEAD_DIM], XD, kind="ExternalInput").ap()
    wkT = nc.dram_tensor("wkT", [128, ND * HEAD_DIM], XD, kind="ExternalInput").ap()
    wvT = nc.dram_tensor("wvT", [128, ND * HEAD_DIM], XD, kind="ExternalInput").ap()
    woT = nc.dram_tensor("woT", [128, HQ * DIM], BF16, kind="ExternalInput").ap()
    mask1 = nc.dram_tensor("mask1", [128, 128], F32, kind="ExternalInput").ap()
    cq = nc.dram_tensor("cq", [128, T], TD, kind="ExternalInput").ap()
    sq = nc.dram_tensor("sq", [128, T], TD, kind="ExternalInput").ap()
    ck = nc.dram_tensor("ck", [128, T], TD, kind="ExternalInput").ap()
    sk = nc.dram_tensor("sk", [128, T], TD, kind="ExternalInput").ap()
    ident = nc.dram_tensor("ident", [128, 128], BF16, kind="ExternalInput").ap()
    yT = nc.dram_tensor("yT", [DIM, T], BF16, kind="ExternalOutput").ap()

    with tile.TileContext(nc) as tc, ExitStack() as ctx:
        const = ctx.enter_context(tc.tile_pool(name="const", bufs=1))
        wp = ctx.enter_context(tc.tile_pool(name="wp", bufs=4))
        qtp = ctx.enter_context(tc.tile_pool(name="qtp", bufs=4))
        rt = ctx.enter_context(tc.tile_pool(name="rt", bufs=1))
        sm = ctx.enter_context(tc.tile_pool(name="sm", bufs=2))
        yp = ctx.enter_context(tc.tile_pool(name="yp", bufs=2))
        ps = ctx.enter_context(tc.tile_pool(name="ps", bufs=7, space=PSUM))
        wps = ctx.enter_context(tc.tile_pool(name="wps", bufs=1, space=PSUM))

        warm_w = const.tile([128, 128], BF16, tag="warm_w")
        nc.vector.memset(warm_w[:], 0.0)
        warm_x = const.tile([128, T], BF16, tag="warm_x")
        nc.vector.memset(warm_x[:], 0.0)
        ps_warm = wps.tile([128, T], F32, tag="wps")
        for _ in range(10):
            nc.tensor.matmul(ps_warm[:], warm_w[:], warm_x[:], start=True, stop=True)

        wk_sb = wp.tile([128, ND * HEAD_DIM], XD, tag="w", name="wk")
        nc.sync.dma_start(wk_sb[:], wkT)
        wv_sb = wp.tile([128, ND * HEAD_DIM], XD, tag="w", name="wv")
        nc.scalar.dma_start(wv_sb[:], wvT)

        x_tiles = [None] * len(XGROUPS)

        def load_x(gi, eng):
            j0, gd = XG_COL[gi]
            xg = const.tile([128, gd * T], XD, tag=f"x{gi}", name=f"x{gi}")
            eng.dma_start(xg[:], xT[:, j0 * T : (j0 + gd) * T])
            x_tiles[gi] = xg

        wq_tiles = [None] * HQ

        def load_wq(h, eng):
            wqt = wp.tile([128, ND * HEAD_DIM], XD, tag="w", name=f"wq{h}")
            eng.dma_start(wqt[:], wqT[:, h * DIM : (h + 1) * DIM])
            wq_tiles[h] = wqt

        load_x(0, nc.sync)
        load_wq(0, nc.scalar)
        load_wq(1, nc.sync)
        load_x(1, nc.scalar)
        load_wq(2, nc.sync)
        load_x(2, nc.scalar)
        load_wq(3, nc.sync)
        ident_sb = const.tile([128, 128], BF16, tag="ident")
        nc.scalar.dma_start(ident_sb[:], ident)
        ck_sb = const.tile([128, T], TD, tag="ck")
        nc.scalar.dma_start(ck_sb[:], ck)
        sk_sb = const.tile([128, T], TD, tag="sk")
        nc.scalar.dma_start(sk_sb[:], sk)
        cq_sb = const.tile([128, T], TD, tag="cq")
        nc.scalar.dma_start(cq_sb[:], cq)
        sq_sb = const.tile([128, T], TD, tag="sq")
        nc.scalar.dma_start(sq_sb[:], sq)
        mask_sb = const.tile([128, 128], F32, tag="mask")
        nc.scalar.dma_start(mask_sb[:], mask1)
        for gi in range(3, len(XGROUPS)):
            load_x(gi, nc.scalar if gi % 2 == 0 else nc.sync)
        wo_sb = const.tile([128, HQ * DIM], BF16, tag="wo")
        nc.sync.dma_start(wo_sb[:, : 2 * DIM], woT[:, : 2 * DIM])
        nc.scalar.dma_start(wo_sb[:, 2 * DIM :], woT[:, 2 * DIM :])

        kT_sb = const.tile([128, T], QD, tag="kT")
        vT_sb = const.tile([128, T], BF16, tag="vT")
        v_sb = const.tile([128, BSZ * HEAD_DIM], BF16, tag="v")
        oT_sb = const.tile([128, HQ * T], BF16, tag="oT")

        def xslice(j):
            for i, (jj0, gd) in enumerate(XG_COL):
                if jj0 <= j < jj0 + gd:
                    return x_tiles[i][:, (j - jj0) * T : (j - jj0 + 1) * T]
            raise AssertionError(j)

        def rope(dst_ap, pssrc, ctab, stab):
            swp = rt.tile([128, T], F32, tag="swp")
            nc.scalar.copy(swp[0:64, :], pssrc[64:128, :])
            nc.scalar.copy(swp[64:128, :], pssrc[0:64, :])
            prod = rt.tile([128, T], F32, tag="prod")
            nc.vector.tensor_mul(prod[:], pssrc[:], ctab)
            nc.vector.tensor_mul(swp[:], swp[:], stab)
            nc.vector.tensor_add(dst_ap, prod[:], swp[:])

        ps_k = ps.tile([128, T], F32, tag="ps")
        ps_v = ps.tile([128, T], F32, tag="ps")
        ps_q = [None] * HQ
        NSW = 2
        for h in range(NSW):
            ps_q[h] = ps.tile([128, T], F32, tag="ps", name=f"ps_q{h}")
        for j in range(ND):
            st, sp = (j == 0), (j == ND - 1)
            xr = xslice(j)
            js = slice(j * HEAD_DIM, (j + 1) * HEAD_DIM)
            nc.tensor.matmul(ps_k[:], wk_sb[:, js], xr, start=st, stop=sp)
            nc.tensor.matmul(ps_v[:], wv_sb[:, js], xr, start=st, stop=sp)
            for h in range(NSW):
                nc.tensor.matmul(ps_q[h][:], wq_tiles[h][:, js], xr, start=st, stop=sp)

        rope(kT_sb[:], ps_k[:], ck_sb[:], sk_sb[:])
        qts = {}
        for h in range(NSW):
            qts[h] = qtp.tile([128, T], QD, tag="qT", name=f"qT{h}")
            rope(qts[h][:], ps_q[h][:], cq_sb[:], sq_sb[:])

        def q_sweep(h):
            ps_qh = ps.tile([128, T], F32, tag="ps", name=f"ps_q{h}")
            for j in range(ND):
                st, sp = (j == 0), (j == ND - 1)
                js = slice(j * HEAD_DIM, (j + 1) * HEAD_DIM)
                nc.tensor.matmul(
                    ps_qh[:], wq_tiles[h][:, js], xslice(j), start=st, stop=sp
                )
            qt = qtp.tile([128, T], QD, tag="qT", name=f"qT{h}")
            rope(qt[:], ps_qh[:], cq_sb[:], sq_sb[:])
            return qt

        def keep_warm(n=2):
            for _ in range(n):
                nc.tensor.matmul(
                    ps_warm[:], warm_w[:], warm_x[:], start=True, stop=True
                )

        def att_scores(h, qt):
            ps_s = ps.tile([128, T], F32, tag="ps", name=f"ps_s{h}")
            for b in range(BSZ):
                bs = slice(b * 128, (b + 1) * 128)
                nc.tensor.matmul(
                    ps_s[:, bs], qt[:, bs], kT_sb[:, bs], start=True, stop=True
                )
            s_sb = sm.tile([128, T], F32, tag="s", name=f"s{h}")
            nmx = sm.tile([128, BSZ], F32, tag="nmx", name=f"nmx{h}")
            den = sm.tile([128, BSZ], F32, tag="den", name=f"den{h}")
            rden = sm.tile([128, BSZ], F32, tag="rden", name=f"rden{h}")
            p_sb = sm.tile([128, T], BF16, tag="p", name=f"p{h}")
            for b in range(BSZ):
                bs = slice(b * 128, (b + 1) * 128)
                nc.vector.tensor_add(s_sb[:, bs], ps_s[:, bs], mask_sb[:])
                nc.vector.reduce_max(
                    nmx[:, b : b + 1], s_sb[:, bs], axis=AX.X, negate=True
                )
                nc.scalar.activation(
                    p_sb[:, bs],
                    s_sb[:, bs],
                    ACTF.Exp,
                    bias=nmx[:, b : b + 1],
                    accum_out=den[:, b : b + 1],
                )
            nc.vector.reciprocal(rden[:], den[:])
            for b in range(BSZ):
                bs = slice(b * 128, (b + 1) * 128)
                nc.vector.tensor_scalar_mul(p_sb[:, bs], p_sb[:, bs], rden[:, b : b + 1])
            return p_sb

        def att_ptrans(h, p_sb):
            ps_pt = ps.tile([128, T], BF16, tag="ps", name=f"ps_pt{h}")
            for b in range(BSZ):
                bs = slice(b * 128, (b + 1) * 128)
                nc.tensor.transpose(ps_pt[:, bs], p_sb[:, bs], ident_sb[:])
            pt_sb = sm.tile([128, T], BF16, tag="pt", name=f"pt{h}")
            nc.scalar.copy(pt_sb[:], ps_pt[:])
            return pt_sb

        def att_pv(h, pt_sb):
            ps_o = ps.tile([128, T], F32, tag="ps", name=f"ps_o{h}")
            for b in range(BSZ):
                bs = slice(b * 128, (b + 1) * 128)
                nc.tensor.matmul(
                    ps_o[:, bs], v_sb[:, bs], pt_sb[:, bs], start=True, stop=True
                )
            if h % 2 == 0:
                nc.vector.tensor_copy(oT_sb[:, h * T : (h + 1) * T], ps_o[:])
            else:
                nc.scalar.copy(oT_sb[:, h * T : (h + 1) * T], ps_o[:])

        probs = {}
        qts[2] = q_sweep(2)
        nc.scalar.copy(vT_sb[:], ps_v[:])
        for b in range(BSZ):
            bs = slice(b * 128, (b + 1) * 128)
            ps_t = ps.tile([128, T], BF16, tag="ps")
            nc.tensor.transpose(ps_t[:, 0:128], vT_sb[:, bs], ident_sb[:])
            nc.vector.tensor_copy(v_sb[:, bs], ps_t[:, 0:128])
        probs[0] = att_scores(0, qts[0])
        probs[1] = att_scores(1, qts[1])
        qts[3] = q_sweep(3)
        att_pv(0, probs[0])
        probs[2] = att_scores(2, qts[2])
        att_pv(1, probs[1])
        keep_warm(2)
        probs[3] = att_scores(3, qts[3])
        att_pv(2, probs[2])
        keep_warm(2)
        att_pv(3, probs[3])

        for dt in range(ND):
            ps_y = ps.tile([128, T], F32, tag="ps", name=f"ps_y{dt}")
            for j in range(HQ):
                nc.tensor.matmul(
                    ps_y[:],
                    wo_sb[:, j * DIM + dt * 128 : j * DIM + (dt + 1) * 128],
                    oT_sb[:, j * T : (j + 1) * T],
                    start=(j == 0),
                    stop=(j == HQ - 1),
                )
            y_sb = yp.tile([128, T], BF16, tag="y", name=f"y{dt}")
            if dt % 2 == 0:
                nc.vector.tensor_copy(y_sb[:], ps_y[:])
                nc.sync.dma_start(yT[dt * 128 : (dt + 1) * 128, :], y_sb[:])
            else:
                nc.scalar.copy(y_sb[:], ps_y[:])
                nc.scalar.dma_start(yT[dt * 128 : (dt + 1) * 128, :], y_sb[:])

    nc.compile()
    return nc


def get_nc(fast: bool):
    key = "nc_fast" if fast else "nc_robust"
    if key not in _STATE:
        _STATE[key] = _build_nc_fast() if fast else _build_nc_robust()
    return _STATE[key]


def _prep_in_maps(x, wq, wk, wv, wo, freqs_cos, freqs_sin, mask, fast):
    f32 = np.float32
    bf16 = ml_dtypes.bfloat16
    xd = bf16 if fast else f32
    x = np.asarray(x, f32)
    wq = np.asarray(wq, f32)
    wk = np.asarray(wk, f32)
    wv = np.asarray(wv, f32)
    wo = np.asarray(wo, f32)
    fc = np.asarray(freqs_cos, f32)
    fs = np.asarray(freqs_sin, f32)
    mask = np.asarray(mask, f32)

    # even features first, then odd: (2i, 2i+1) pairs -> (i, i+64)
    perm = np.concatenate([np.arange(0, HEAD_DIM, 2), np.arange(1, HEAD_DIM, 2)])
    wqp = wq.reshape(N_HEADS, HEAD_DIM, DIM)[:, perm, :].reshape(DIM, DIM)
    wkp = wk.reshape(N_KV_HEADS, HEAD_DIM, DIM)[:, perm, :].reshape(
        N_KV_HEADS * HEAD_DIM, DIM
    )

    def sw_x(xmat):  # [T, DIM] -> [128, ND*T]
        return np.ascontiguousarray(
            xmat.T.reshape(ND, 128, T).transpose(1, 0, 2).reshape(128, ND * T)
        )

    def sw_w(wmat):  # [E(128), DIM] -> [128, ND*E]
        E = wmat.shape[0]
        return np.ascontiguousarray(
            wmat.T.reshape(ND, 128, E).transpose(1, 0, 2).reshape(128, ND * E)
        )

    xT = sw_x(x.reshape(T, DIM)).astype(xd)
    C0 = np.vstack([fc.T, fc.T])  # [128, 128]: row p -> cos[t, p % 64]
    S0 = np.vstack([-fs.T, fs.T])
    td = bf16 if fast else f32
    cq = np.ascontiguousarray(np.tile(C0 * SCALE, (1, BSZ))).astype(td)
    sq = np.ascontiguousarray(np.tile(S0 * SCALE, (1, BSZ))).astype(td)
    ck = np.ascontiguousarray(np.tile(C0, (1, BSZ))).astype(td)
    sk = np.ascontiguousarray(np.tile(S0, (1, BSZ))).astype(td)
    ident = np.eye(128, dtype=bf16)

    in_maps = []
    for c in range(NCORES):
        qrows = slice(c * EQ, (c + 1) * EQ)
        krows = slice(c * HEAD_DIM, (c + 1) * HEAD_DIM)
        wq_heads = [
            sw_w(wqp[c * EQ + h * HEAD_DIM : c * EQ + (h + 1) * HEAD_DIM, :])
            for h in range(HQ)
        ]
        wo_sw = np.ascontiguousarray(
            wo[:, qrows].T.reshape(HQ, 128, DIM).transpose(1, 0, 2).reshape(128, HQ * DIM)
        ).astype(bf16)
        m = {
            "xT": xT,
            "wqT": np.ascontiguousarray(np.concatenate(wq_heads, axis=1)).astype(xd),
            "wkT": sw_w(wkp[krows, :]).astype(xd),
            "wvT": sw_w(wv[krows, :]).astype(xd),
            "woT": wo_sw,
            "cq": cq,
            "sq": sq,
            "ck": ck,
            "sk": sk,
            "ident": ident,
        }
        if fast:
            m["mask4"] = np.ascontiguousarray(np.tile(mask[0, 0], (1, BSZ))).astype(f32)
        else:
            m["mask1"] = np.ascontiguousarray(mask[0, 0])
        in_maps.append(m)
    return in_maps


def _pick_fast(x, wq):
    """bf16 q/k only when softmax logits are smooth (score sigma small)."""
    sx = float(np.asarray(x, np.float32).std())
    sw = float(np.asarray(wq, np.float32).std())
    sigma = sx * sw * math.sqrt(DIM * HEAD_DIM) * SCALE
    return sigma < 8.0


def kernel(
    x,
    wq,
    wk,
    wv,
    wo,
    cache_k,
    cache_v,
    freqs_cos,
    freqs_sin,
    mask,
    start_pos,
    *,
    trace=False,
    trace_kwargs=None,
):
    global LAST_RESULT
    sp = int(np.asarray(start_pos))
    assert sp == 0, f"kernel specialized for start_pos=0, got {sp}"

    fast = _pick_fast(x, wq)
    in_maps = _prep_in_maps(x, wq, wk, wv, wo, freqs_cos, freqs_sin, mask, fast)
    nc = get_nc(fast)
    res = run_bass_kernel_spmd(
        nc,
        in_maps,
        core_ids=list(range(NCORES)),
        trace=trace,
        **(trace_kwargs or {}),
    )
    LAST_RESULT = res
    acc = np.zeros((DIM, T), np.float32)
    for c in range(NCORES):
        acc += res.results[c]["yT"].astype(np.float32)
    return np.ascontiguousarray(acc.T).reshape(BSZ, SEQLEN, DIM)
